# revision 1
# baseline (speedup 1.0000x reference)
"""Trainium2 Bass kernel for the YAT MixerBlock (nn_MixerBlock_12524124635797).

Strategy: pure data-parallel over batch (64 -> 8 per core). Each core runs
the full mixer block for its 8 batch elements.

Per-core dataflow (all GEMMs fp16 inputs, fp32 PSUM accumulation):
  Token stage (per batch b, x_b is (196p, 768c)):
    dot1 (384t-part, 768c-free) = twT.T @ x_b            [PE]
    den  = wn_t[t] + xn[c] - 2*dot1 + eps                [DVE affine_then_add]
    rec  = 1/den                                         [DVE reciprocal_approx_fast]
    sq   = (dot1 + tb[t])^2                              [ACT Square, bias slot]
    h1   = sq * rec  (fp16)                              [GPSIMD mult; scale_t folded into w2]
    x2T (768c-part, 196p-free) = h1.T@w2sT + x_b.T@I196 + ones.T@b2row   [PE, shortcut+bias
                                                          folded in as extra K rows]
  Channel stage (rows = (b,p) flattened, 1568 per core):
    xn2b (128, rows) = ones.T @ (x2T*x2T)                [PE broadcast of row norms]
    for row-block rb, for m-chunk mc (24 chunks of 3072):
      dot2 (128m-part, rows-free) = cwT.T @ x2T          [PE]
      den2/rec2/sq2/h2 as above (wn_c, cb per-partition) [DVE/ACT/GPSIMD]
      out_psum(rows-part, 768c) += h2.T @ w4sT[mc]       [PE]
    out_psum += x2T.T @ I768 + ones.T @ b4row            [PE, shortcut+bias]
    out (rows, 768) fp32 -> DRAM                         [ACT copy + DMA]
"""

import numpy as np

import concourse.bass as bass
import concourse.bacc as bacc
import concourse.mybir as mybir
from concourse import bass_utils
from concourse import tile

F16 = mybir.dt.float16
F32 = mybir.dt.float32
AF = mybir.ActivationFunctionType

EPS = 0.1
B, P, C, T, M3 = 64, 196, 768, 384, 3072
NCORES = 8
BL = B // NCORES          # 8 batches per core
ROWS = BL * P             # 1568 rows per core
ROWSP = 1664              # ROWS padded to a multiple of 128
RB = 256                  # row-block size for the channel stage (2 psum chunks)


def _ceil_div(a, b):
    return (a + b - 1) // b


def _n_slices(n, step=512):
    """Split [0, n) into matmul-legal free-dim slices (<=512, bank-aligned)."""
    out = []
    o = 0
    while o < n:
        out.append((o, min(step, n - o)))
        o += step
    return out


def build_program():
    nc = bacc.Bacc(
        "TRN2",
        target_bir_lowering=False,
        debug=False,
        enable_asserts=False,
        num_devices=NCORES,
    )

    # ---- DRAM I/O ----
    d = {}
    d["xa"] = nc.dram_tensor("xa", [BL, 128, C], F16, kind="ExternalInput").ap()
    d["xb"] = nc.dram_tensor("xb", [BL, 128, C], F16, kind="ExternalInput").ap()
    d["twT"] = nc.dram_tensor("twT", [128, 2, T], F16, kind="ExternalInput").ap()
    d["w2sT"] = nc.dram_tensor("w2sT", [128, 3, P], F16, kind="ExternalInput").ap()
    d["i196"] = nc.dram_tensor("i196", [128, 2, P], F16, kind="ExternalInput").ap()
    d["b2r"] = nc.dram_tensor("b2r", [1, P], F16, kind="ExternalInput").ap()
    d["cwT"] = nc.dram_tensor("cwT", [128, 6, M3], F16, kind="ExternalInput").ap()
    d["w4sT"] = nc.dram_tensor("w4sT", [128, 24, C], F16, kind="ExternalInput").ap()
    d["b4r"] = nc.dram_tensor("b4r", [1, C], F16, kind="ExternalInput").ap()
    d["wnt"] = nc.dram_tensor("wnt", [128, 3], F32, kind="ExternalInput").ap()
    d["tbc"] = nc.dram_tensor("tbc", [128, 3], F32, kind="ExternalInput").ap()
    d["wnc"] = nc.dram_tensor("wnc", [128, 24], F32, kind="ExternalInput").ap()
    d["cbc"] = nc.dram_tensor("cbc", [128, 24], F32, kind="ExternalInput").ap()
    out_dram = nc.dram_tensor("out", [ROWS, C], F32, kind="ExternalOutput").ap()

    with tile.TileContext(nc) as tc:
        with tc.tile_pool(name="consts", bufs=1) as cp:
            # Resident constants / persistent activations.
            twT = cp.tile([128, 2, T], F16)
            w2sT = cp.tile([128, 3, P], F16)
            i196 = cp.tile([128, 2, P], F16)
            b2r = cp.tile([128, P], F16)
            cwT = cp.tile([128, 6, M3], F16)
            w4sT = cp.tile([128, 24, C], F16)
            b4r = cp.tile([128, C], F16)
            wnt = cp.tile([128, 3], F32)
            tbc = cp.tile([128, 3], F32)
            wnc = cp.tile([128, 24], F32)
            cbc = cp.tile([128, 24], F32)
            ones = cp.tile([128, 128], F16)
            # Free dim padded to a multiple of 128 so the tail row-block's
            # 128-col DMA transpose reads stay in bounds (garbage cols unused).
            x2T = cp.tile([128, 6, ROWSP], F16)
            xn2b = cp.tile([128, ROWS], F32)

            # x input first (token stage's critical path) as two big strided
            # DMAs, then small token constants, all on the sync queue; the big
            # channel weights go on the scalar-engine HWDGE queue so they
            # don't block the token stage.
            # Per-batch x tiles: separate tiles so batch 0's consumers only
            # wait on batch 0's DMA. Startup-critical loads go first on sync;
            # big channel weights on the scalar queue.
            xbs = []
            nc.sync.dma_start(twT[:], d["twT"])
            for b in range(BL):
                xb = cp.tile([128, 2, C], F16, name=f"xb{b}")
                nc.sync.dma_start(xb[:, 0, :], d["xa"][b])
                nc.sync.dma_start(xb[0:68, 1, :], d["xb"][b, 0:68, :])
                xbs.append(xb)
                if b == 0:
                    nc.sync.dma_start(w2sT[:], d["w2sT"])
                    nc.sync.dma_start(i196[:], d["i196"])
                    nc.sync.dma_start(b2r[0:1, :], d["b2r"])
                    nc.sync.dma_start(wnt[:], d["wnt"])
                    nc.sync.dma_start(tbc[:], d["tbc"])
            nc.sync.dma_start(wnc[:], d["wnc"])
            nc.sync.dma_start(cbc[:], d["cbc"])
            nc.scalar.dma_start(cwT[:], d["cwT"])
            nc.scalar.dma_start(w4sT[:], d["w4sT"])
            nc.scalar.dma_start(b4r[0:1, :], d["b4r"])
            nc.vector.memset(ones[:], 1.0)
            nc.vector.memset(x2T[:, :, ROWS:ROWSP], 0.0)

            # ================= Token stage =================
            with (
                tc.tile_pool(name="tok_sbuf", bufs=2) as tp,
                tc.tile_pool(name="tok_psum", bufs=1, space="PSUM") as pp,
            ):
                for b in range(BL):
                    r0 = b * P
                    xb = xbs[b]

                    # dot1 first: it only needs twT + x, so the PE can start
                    # before the norm chain is ready.
                    dot1s = []
                    for tcn in range(3):
                        ps_dot1 = pp.tile(
                            [128, C], F32, tag="ps_dot1", bufs=2, name="ps_dot1"
                        )
                        for kc, kn in ((0, 128), (1, 68)):
                            for no, nn_ in _n_slices(C):
                                nc.tensor.matmul(
                                    ps_dot1[:, no : no + nn_],
                                    twT[0:kn, kc, tcn * 128 : (tcn + 1) * 128],
                                    xb[0:kn, kc, no : no + nn_],
                                    start=(kc == 0),
                                    stop=(kc == 1),
                                )
                        dot1s.append(ps_dot1)

                    # x-norm broadcast tile: xnb[q, c] = sum_p x[p, c]^2
                    xsq = tp.tile([128, 2, C], F16, tag="xsq")
                    nc.vector.tensor_mul(xsq[:, 0, :], xb[:, 0, :], xb[:, 0, :])
                    nc.vector.tensor_mul(
                        xsq[0:68, 1, :], xb[0:68, 1, :], xb[0:68, 1, :]
                    )
                    ps_xnb = pp.tile([128, C], F32, tag="ps_xnb", bufs=1)
                    for no, nn_ in _n_slices(C):
                        nc.tensor.matmul(
                            ps_xnb[:, no : no + nn_],
                            ones[:, :],
                            xsq[:, 0, no : no + nn_],
                            start=True,
                            stop=False,
                        )
                        nc.tensor.matmul(
                            ps_xnb[:, no : no + nn_],
                            ones[0:68, :],
                            xsq[0:68, 1, no : no + nn_],
                            start=False,
                            stop=True,
                        )
                    xnb = tp.tile([128, C], F32, tag="xnb")
                    nc.scalar.copy(xnb[:], ps_xnb[:])

                    h1 = tp.tile([128, 3, C], F16, tag="h1")
                    for tcn in range(3):
                        ps_dot1 = dot1s[tcn]
                        den = tp.tile([128, C], F32, tag="den")
                        nc.vector.affine_then_add(
                            den[:], ps_dot1[:], xnb[:],
                            scale=-2.0, bias=wnt[:, tcn : tcn + 1],
                        )
                        rec = tp.tile([128, C], F32, tag="rec")
                        nc.vector.reciprocal_approx_fast(rec[:], den[:])
                        sq = tp.tile([128, C], F32, tag="sq")
                        nc.scalar.activation(
                            sq[:], ps_dot1[:], AF.Square, bias=tbc[:, tcn : tcn + 1]
                        )
                        nc.gpsimd.tensor_mul(h1[:, tcn, :], sq[:], rec[:])

                    # token linear + shortcut + bias -> x2T columns for batch b
                    for mc in range(6):
                        ps_x2 = pp.tile([128, P], F32, tag="ps_x2", bufs=2)
                        for kc in range(3):
                            nc.tensor.matmul(
                                ps_x2[:],
                                h1[:, kc, mc * 128 : (mc + 1) * 128],
                                w2sT[:, kc, :],
                                start=(kc == 0),
                                stop=False,
                            )
                        for kc, kn in ((0, 128), (1, 68)):
                            nc.tensor.matmul(
                                ps_x2[:],
                                xb[0:kn, kc, mc * 128 : (mc + 1) * 128],
                                i196[0:kn, kc, :],
                                start=False,
                                stop=False,
                            )
                        nc.tensor.matmul(
                            ps_x2[:],
                            ones[0:1, :],
                            b2r[0:1, :],
                            start=False,
                            stop=True,
                        )
                        nc.scalar.copy(x2T[:, mc, r0 : r0 + P], ps_x2[:])

            # ================= Channel-stage row norms =================
            with (
                tc.tile_pool(name="xn_sbuf", bufs=1) as xp,
                tc.tile_pool(name="xn_psum", bufs=1, space="PSUM") as xpp,
            ):
                ps_xn2 = xpp.tile([128, ROWS], F32)
                for kc in range(6):
                    x2sq = xp.tile([128, ROWS], F16, tag="x2sq", bufs=2)
                    nc.vector.tensor_mul(x2sq[:], x2T[:, kc, 0:ROWS], x2T[:, kc, 0:ROWS])
                    for no, nn_ in _n_slices(ROWS):
                        nc.tensor.matmul(
                            ps_xn2[:, no : no + nn_],
                            ones[:, :],
                            x2sq[:, no : no + nn_],
                            start=(kc == 0),
                            stop=(kc == 5),
                        )
                nc.scalar.copy(xn2b[:], ps_xn2[:])

            # ================= Channel stage =================
            with (
                tc.tile_pool(name="ch_sbuf", bufs=2) as chp,
                tc.tile_pool(name="ch_psum", bufs=1, space="PSUM") as cpp,
            ):
                for r0 in range(0, ROWS, RB):
                    rn = min(RB, ROWS - r0)
                    nsub = _ceil_div(rn, 128)
                    po = [
                        cpp.tile([128, C], F32, tag=f"po{s}", bufs=1, name=f"po{s}")
                        for s in range(nsub)
                    ]
                    for mc in range(24):
                        ps_d2 = cpp.tile([128, RB], F32, tag="ps_d2", bufs=4)
                        for kc in range(6):
                            nc.tensor.matmul(
                                ps_d2[:, 0:rn],
                                cwT[:, kc, mc * 128 : (mc + 1) * 128],
                                x2T[:, kc, r0 : r0 + rn],
                                start=(kc == 0),
                                stop=(kc == 5),
                            )
                        den2 = chp.tile([128, RB], F32, tag="den2", bufs=4)
                        nc.vector.affine_then_add(
                            den2[:, 0:rn], ps_d2[:, 0:rn], xn2b[:, r0 : r0 + rn],
                            scale=-2.0, bias=wnc[:, mc : mc + 1],
                        )
                        rec2 = chp.tile([128, RB], F32, tag="rec2", bufs=4)
                        nc.vector.reciprocal_approx_fast(rec2[:, 0:rn], den2[:, 0:rn])
                        sq2 = chp.tile([128, RB], F32, tag="sq2", bufs=4)
                        nc.scalar.activation(
                            sq2[:, 0:rn], ps_d2[:, 0:rn], AF.Square,
                            bias=cbc[:, mc : mc + 1],
                        )
                        h2 = chp.tile([128, RB], F16, tag="h2", bufs=4)
                        # Alternate the multiply between GPSIMD and DVE so the
                        # last link of the yat chain isn't serialized on one
                        # engine's FIFO.
                        mul_eng = nc.gpsimd if mc % 3 else nc.vector
                        mul_eng.tensor_mul(h2[:, 0:rn], sq2[:, 0:rn], rec2[:, 0:rn])

                        for s in range(nsub):
                            sn = min(128, rn - s * 128)
                            for no, nn_ in _n_slices(C):
                                nc.tensor.matmul(
                                    po[s][0:sn, no : no + nn_],
                                    h2[:, s * 128 : s * 128 + sn],
                                    w4sT[:, mc, no : no + nn_],
                                    start=(mc == 0),
                                    stop=False,
                                )
                    # bias b4 row, then shortcut x2 added via DVE from a
                    # DMA-transposed copy of x2T (cheaper than routing the
                    # identity through the PE).
                    for s in range(nsub):
                        sn = min(128, rn - s * 128)
                        rs = r0 + s * 128
                        for no, nn_ in _n_slices(C):
                            nc.tensor.matmul(
                                po[s][0:sn, no : no + nn_],
                                ones[0:1, 0:sn],
                                b4r[0:1, no : no + nn_],
                                start=False,
                                stop=True,
                            )
                        x2row = chp.tile([128, 6, 128], F16, tag="x2row", bufs=3)
                        for kc in range(6):
                            # Always a full 128-col source block (x2T free dim
                            # is padded); extra rows of x2row are unused.
                            nc.sync.dma_start_transpose(
                                x2row[:, kc, :], x2T[:, kc, rs : rs + 128]
                            )
                        osb = chp.tile([128, C], F32, tag="osb", bufs=3)
                        nc.vector.tensor_add(
                            osb[0:sn, :],
                            po[s][0:sn, :],
                            x2row[0:sn, :, :].rearrange("p a b -> p (a b)"),
                        )
                        nc.sync.dma_start(out_dram[rs : rs + sn, :], osb[0:sn, :])

    nc.compile()
    return nc


def _pack_kpn(w, n_chunks):
    """(K, N) fp32 -> (128, n_chunks, N) fp16 with zero padding of K."""
    k, n = w.shape
    out = np.zeros((n_chunks * 128, n), np.float16)
    out[:k] = w.astype(np.float16)
    return np.ascontiguousarray(
        out.reshape(n_chunks, 128, n).transpose(1, 0, 2)
    )


def _pack_col(v, n_chunks):
    """(K,) fp32 -> (128, n_chunks) fp32 column chunks."""
    out = np.zeros((n_chunks * 128,), np.float32)
    out[: v.shape[0]] = v.astype(np.float32)
    return np.ascontiguousarray(out.reshape(n_chunks, 128).T)


_PROGRAM = None


def _get_program():
    global _PROGRAM
    if _PROGRAM is None:
        _PROGRAM = build_program()
    return _PROGRAM


def kernel(x, tw, tb, t_alpha, w2, b2, cw, cb, c_alpha, w4, b4, _trace=False):
    x = np.asarray(x, np.float32)
    tw = np.asarray(tw, np.float32)
    tb = np.asarray(tb, np.float32)
    w2 = np.asarray(w2, np.float32)
    b2 = np.asarray(b2, np.float32)
    cw = np.asarray(cw, np.float32)
    cb = np.asarray(cb, np.float32)
    w4 = np.asarray(w4, np.float32)
    b4 = np.asarray(b4, np.float32)

    # YAT output scales (exactly as the reference computes them), folded into
    # the following linear layers' weights and biases' stays separate.
    scale_t = np.float32(np.sqrt(np.float32(T / np.log(T + 1.0)))) ** np.asarray(
        t_alpha, np.float32
    )[0]
    scale_c = np.float32(np.sqrt(np.float32(M3 / np.log(M3 + 1.0)))) ** np.asarray(
        c_alpha, np.float32
    )[0]
    w2s = (w2 * scale_t).astype(np.float32)   # (P, T)
    w4s = (w4 * scale_c).astype(np.float32)   # (C, M3)

    shared = {
        "twT": _pack_kpn(tw.T, 2),                       # (196,384) -> (128,2,384)
        "w2sT": _pack_kpn(w2s.T, 3),                     # (384,196) -> (128,3,196)
        "i196": _pack_kpn(np.eye(P, dtype=np.float32), 2),
        "b2r": b2.astype(np.float16).reshape(1, P),
        "cwT": _pack_kpn(cw.T, 6),                       # (768,3072)
        "w4sT": _pack_kpn(w4s.T, 24),                    # (3072,768)
        "b4r": b4.astype(np.float16).reshape(1, C),
        "wnt": _pack_col((tw.astype(np.float32) ** 2).sum(1) + EPS, 3),
        "tbc": _pack_col(tb, 3),
        "wnc": _pack_col((cw.astype(np.float32) ** 2).sum(1) + EPS, 24),
        "cbc": _pack_col(cb, 24),
    }
    x16 = x.astype(np.float16).reshape(NCORES, BL, P, C)
    xa = np.ascontiguousarray(x16[:, :, 0:128, :])
    xbp = np.zeros((NCORES, BL, 128, C), np.float16)
    xbp[:, :, 0:68] = x16[:, :, 128:P, :]
    in_maps = [dict(shared, xa=xa[c], xb=xbp[c]) for c in range(NCORES)]

    nc = _get_program()
    kwargs = {}
    if _trace:
        import shutil

        shutil.rmtree("/tmp/bass_ntff", ignore_errors=True)
        import os

        os.makedirs("/tmp/bass_ntff", exist_ok=True)
        kwargs["tmpdir"] = "/tmp/bass_ntff"
    res = bass_utils.run_bass_kernel_spmd(
        nc, in_maps, core_ids=list(range(NCORES)), trace=_trace, **kwargs
    )
    out = np.concatenate([res.results[c]["out"] for c in range(NCORES)], axis=0)
    out = out.reshape(B, P, C).astype(np.float32)
    if _trace:
        kernel.last_results = res
    return out



# revision 3
# speedup vs baseline: 1.5253x; 1.5253x over previous
"""Trainium2 Bass kernel for the YAT MixerBlock (nn_MixerBlock_12524124635797).

Data-parallel over batch (64 -> 8 per core); all four GEMMs run as fp8e4
DoubleRow matmuls (2 K-chunks per instruction = 2x fp16 PE throughput).

Scaling scheme (power-of-2 scales keep fp8/fp16 ranges healthy; exact
compensation happens in fp32 psum / affine ops):
  tw8 = q8(-64*tw), cw8 = q8(-64*cw)       -> psum_dot = -64*dot
  den ops produce 32*den = psum + 32*(wn+eps) + 32*xn   [DVE affine]
  rec = 1/(32*den)                                       [DVE recip]
  sq  = Square(-0.5*psum + 32*bias) = 1024*(dot+b)^2     [ACT]
  h8  = sq*rec = 32*h  (fp8)                             [Pool mul]
  w2s8 = q8(2*scale_t*w2), w4s8 = q8(2*scale_c*w4)
  x2T = psum/(32*2) + (x.T + b2)                         [DVE affine]
  out = psum/(32*2) + x2T  (+64*b4 folded into the GEMM via fp8 row)

Per-core layout: token stage works per batch in (p-part, c) orientation;
channel stage works entirely transposed (c-part, rows=b*196+p free), the
final output is written as outT (768 x 1568) and transposed on host.
"""

import numpy as np
import ml_dtypes

import concourse.bass as bass
import concourse.bacc as bacc
import concourse.mybir as mybir
from concourse import bass_utils
from concourse import tile

F8 = mybir.dt.float8e4
F16 = mybir.dt.float16
F32 = mybir.dt.float32
AF = mybir.ActivationFunctionType
DR = mybir.MatmulPerfMode.DoubleRow

EPS = 0.1
B, P, C, T, M3 = 64, 196, 768, 384, 3072
NCORES = 8
BL = B // NCORES          # 8 batches per core
ROWS = BL * P             # 1568 rows per core
RB = 392                  # channel row-block (4 equal blocks)
NBLK = ROWS // RB


def build_program():
    nc = bacc.Bacc(
        "TRN2",
        target_bir_lowering=False,
        debug=False,
        enable_asserts=False,
        num_devices=NCORES,
    )

    d = {}
    d["x8"] = nc.dram_tensor("x8", [BL, 128, 2, C], F8, kind="ExternalInput").ap()
    d["tw8"] = nc.dram_tensor("tw8", [128, 2, T], F8, kind="ExternalInput").ap()
    d["xn1s"] = nc.dram_tensor("xn1s", [1, BL * C], F16, kind="ExternalInput").ap()
    d["wntS"] = nc.dram_tensor("wntS", [128, 3], F32, kind="ExternalInput").ap()
    d["tb32"] = nc.dram_tensor("tb32", [128, 3], F32, kind="ExternalInput").ap()
    d["w2s8"] = nc.dram_tensor("w2s8", [128, 4, P], F8, kind="ExternalInput").ap()
    d["xTp"] = nc.dram_tensor("xTp", [128, 6, ROWS], F16, kind="ExternalInput").ap()
    d["cw8"] = nc.dram_tensor("cw8", [128, 6, M3], F8, kind="ExternalInput").ap()
    d["wncS"] = nc.dram_tensor("wncS", [128, 24], F32, kind="ExternalInput").ap()
    d["cb32"] = nc.dram_tensor("cb32", [128, 24], F32, kind="ExternalInput").ap()
    d["w4s8"] = nc.dram_tensor("w4s8", [128, 24, C], F8, kind="ExternalInput").ap()
    d["b4s8"] = nc.dram_tensor("b4s8", [1, C], F8, kind="ExternalInput").ap()
    out_dram = nc.dram_tensor("outT", [C, ROWS], F16, kind="ExternalOutput").ap()

    with tile.TileContext(nc) as tc:
        with tc.tile_pool(name="consts", bufs=1) as cp:
            tw8 = cp.tile([128, 2, T], F8)
            xn1s = cp.tile([1, BL * C], F16)
            wntS = cp.tile([128, 3], F32)
            tb32 = cp.tile([128, 3], F32)
            w2s8 = cp.tile([128, 4, P], F8)
            xTp = cp.tile([128, 6, ROWS], F16)
            cw8 = cp.tile([128, 6, M3], F8)
            wncS = cp.tile([128, 24], F32)
            cb32 = cp.tile([128, 24], F32)
            w4s8 = cp.tile([128, 24, C], F8)
            b4s8 = cp.tile([1, C], F8)
            onecol = cp.tile([1, 128], F16)
            ones32 = cp.tile([128, 128], F16)
            one8 = cp.tile([1, 512], F8)
            x2T = cp.tile([128, 6, ROWS], F16)
            x2T8 = cp.tile([128, 6, ROWS], F8)

            # token-critical loads first on the sync queue
            nc.sync.dma_start(tw8[:], d["tw8"])
            nc.sync.dma_start(xn1s[:], d["xn1s"])
            nc.sync.dma_start(wntS[:], d["wntS"])
            nc.sync.dma_start(tb32[:], d["tb32"])
            nc.sync.dma_start(w2s8[:], d["w2s8"])
            xbs = []
            for b in range(BL):
                xb = cp.tile([128, 2, C], F8, name=f"xb{b}")
                nc.sync.dma_start(xb[:], d["x8"][b])
                xbs.append(xb)
            nc.sync.dma_start(xTp[:], d["xTp"])
            # channel weights on the scalar-engine HWDGE queue
            nc.scalar.dma_start(cw8[:], d["cw8"])
            nc.scalar.dma_start(w4s8[:], d["w4s8"])
            nc.scalar.dma_start(b4s8[:], d["b4s8"])
            nc.sync.dma_start(wncS[:], d["wncS"])
            nc.sync.dma_start(cb32[:], d["cb32"])
            nc.vector.memset(onecol[:], 1.0)
            nc.vector.memset(ones32[:], 32.0)
            nc.vector.memset(one8[:], 1.0)

            # ================= Token stage =================
            with (
                tc.tile_pool(name="tok_sbuf", bufs=2) as tp,
                tc.tile_pool(name="tok_psum", bufs=1, space="PSUM") as pp,
            ):
                for b in range(BL):
                    r0 = b * P
                    xb = xbs[b]

                    # 32*xn broadcast: ones-col (K=1) matmul of the host row
                    ps_xnb = pp.tile([128, C], F32, tag="xnb", bufs=1)
                    for no, nn_ in ((0, 512), (512, 256)):
                        nc.tensor.matmul(
                            ps_xnb[:, no : no + nn_],
                            onecol[0:1, :],
                            xn1s[0:1, b * C + no : b * C + no + nn_],
                            start=True, stop=True,
                        )
                    xnb = tp.tile([128, C], F32, tag="xnbs")
                    nc.scalar.copy(xnb[:], ps_xnb[:])

                    h8 = tp.tile([128, 4, C], F8, tag="h8")
                    # chunk 3 pairs with w2s8's zero chunk; garbage fp8 NaN
                    # bits would still poison 0*NaN -> zero it.
                    nc.gpsimd.memset(h8[:, 3, :], 0.0)
                    for tcn in range(3):
                        ps1 = pp.tile([128, C], F32, tag="ps1", bufs=2)
                        for no, nn_ in ((0, 512), (512, 256)):
                            nc.tensor.matmul(
                                ps1[:, no : no + nn_],
                                tw8[:, 0:2, tcn * 128 : (tcn + 1) * 128],
                                xb[:, 0:2, no : no + nn_],
                                start=True, stop=True, perf_mode=DR,
                            )
                        den = tp.tile([128, C], F32, tag="den")
                        nc.vector.affine_then_add(
                            den[:], ps1[:], xnb[:],
                            scale=1.0, bias=wntS[:, tcn : tcn + 1],
                        )
                        rec = tp.tile([128, C], F32, tag="rec")
                        nc.vector.reciprocal_approx_fast(rec[:], den[:])
                        sq = tp.tile([128, C], F16, tag="sq")
                        nc.scalar.activation(
                            sq[:], ps1[:], AF.Square,
                            bias=tb32[:, tcn : tcn + 1], scale=-0.5,
                        )
                        nc.gpsimd.tensor_mul(h8[:, tcn, :], sq[:], rec[:])

                    for mc in range(6):
                        ps2 = pp.tile([128, P], F32, tag="ps2", bufs=2)
                        for j in range(2):
                            nc.tensor.matmul(
                                ps2[:],
                                h8[:, 2 * j : 2 * j + 2, mc * 128 : (mc + 1) * 128],
                                w2s8[:, 2 * j : 2 * j + 2, :],
                                start=(j == 0), stop=(j == 1), perf_mode=DR,
                            )
                        nc.vector.affine_then_add(
                            x2T[:, mc, r0 : r0 + P], ps2[:],
                            xTp[:, mc, r0 : r0 + P],
                            scale=1.0 / 64.0, bias=0.0,
                        )
                        nc.scalar.copy(
                            x2T8[:, mc, r0 : r0 + P], x2T[:, mc, r0 : r0 + P]
                        )

            # ================= Channel stage =================
            with (
                tc.tile_pool(name="ch_sbuf", bufs=2) as chp,
                tc.tile_pool(name="ch_psum", bufs=1, space="PSUM") as cpp,
            ):
                for blk in range(NBLK):
                    r0 = blk * RB

                    # 32*row-norms: Pool squares + ones(32) matmul
                    ps_xn2 = cpp.tile([128, RB], F32, tag="ps_d2", bufs=2)
                    x2sq = chp.tile([128, 6, RB], F16, tag="x2sq", bufs=2)
                    for kc in range(6):
                        nc.gpsimd.tensor_mul(
                            x2sq[:, kc, :],
                            x2T[:, kc, r0 : r0 + RB],
                            x2T[:, kc, r0 : r0 + RB],
                        )
                        nc.tensor.matmul(
                            ps_xn2[:],
                            ones32[:, :],
                            x2sq[:, kc, :],
                            start=(kc == 0), stop=(kc == 5),
                        )
                    xnb2 = chp.tile([128, RB], F32, tag="xnb2", bufs=2)
                    nc.scalar.copy(xnb2[:], ps_xn2[:])

                    po = [
                        cpp.tile([128, RB], F32, tag=f"po{s}", bufs=1, name=f"po{s}")
                        for s in range(6)
                    ]
                    for j in range(12):
                        h8p = chp.tile([128, 2, RB], F8, tag="h8p", bufs=3)
                        for i in range(2):
                            mc = 2 * j + i
                            ps_d2 = cpp.tile([128, RB], F32, tag="ps_d2", bufs=2)
                            for k in range(3):
                                nc.tensor.matmul(
                                    ps_d2[:],
                                    cw8[:, 2 * k : 2 * k + 2,
                                        mc * 128 : (mc + 1) * 128],
                                    x2T8[:, 2 * k : 2 * k + 2, r0 : r0 + RB],
                                    start=(k == 0), stop=(k == 2), perf_mode=DR,
                                )
                            den2 = chp.tile([128, RB], F32, tag="den2", bufs=4)
                            nc.vector.affine_then_add(
                                den2[:], ps_d2[:], xnb2[:],
                                scale=1.0, bias=wncS[:, mc : mc + 1],
                            )
                            rec2 = chp.tile([128, RB], F32, tag="rec2", bufs=4)
                            nc.vector.reciprocal_approx_fast(rec2[:], den2[:])
                            sq2 = chp.tile([128, RB], F16, tag="sq2", bufs=4)
                            nc.scalar.activation(
                                sq2[:], ps_d2[:], AF.Square,
                                bias=cb32[:, mc : mc + 1], scale=-0.5,
                            )
                            nc.gpsimd.tensor_mul(h8p[:, i, :], sq2[:], rec2[:])
                        for cc in range(6):
                            nc.tensor.matmul(
                                po[cc][:],
                                w4s8[:, 2 * j : 2 * j + 2,
                                     cc * 128 : (cc + 1) * 128],
                                h8p[:, 0:2, :],
                                start=(j == 0), stop=False, perf_mode=DR,
                            )
                    for cc in range(6):
                        # +64*b4 via fp8 K=1 row, closes the accumulation
                        nc.tensor.matmul(
                            po[cc][:],
                            b4s8[0:1, cc * 128 : (cc + 1) * 128],
                            one8[0:1, 0:RB],
                            start=False, stop=True,
                        )
                        o16 = chp.tile([128, RB], F16, tag="o16", bufs=3)
                        nc.vector.affine_then_add(
                            o16[:], po[cc][:], x2T[:, cc, r0 : r0 + RB],
                            scale=1.0 / 64.0, bias=0.0,
                        )
                        nc.sync.dma_start(
                            out_dram[cc * 128 : (cc + 1) * 128, r0 : r0 + RB],
                            o16[:],
                        )

    nc.compile()
    return nc


_Q8 = ml_dtypes.float8_e4m3


def _q8(a):
    return np.asarray(a, np.float32).astype(_Q8)


_PROGRAM = None


def _get_program():
    global _PROGRAM
    if _PROGRAM is None:
        _PROGRAM = build_program()
    return _PROGRAM


def kernel(x, tw, tb, t_alpha, w2, b2, cw, cb, c_alpha, w4, b4, _trace=False):
    x = np.asarray(x, np.float32)
    tw = np.asarray(tw, np.float32)
    tb = np.asarray(tb, np.float32)
    w2 = np.asarray(w2, np.float32)
    b2 = np.asarray(b2, np.float32)
    cw = np.asarray(cw, np.float32)
    cb = np.asarray(cb, np.float32)
    w4 = np.asarray(w4, np.float32)
    b4 = np.asarray(b4, np.float32)

    scale_t = np.float32(np.sqrt(np.float32(T / np.log(T + 1.0)))) ** np.asarray(
        t_alpha, np.float32
    )[0]
    scale_c = np.float32(np.sqrt(np.float32(M3 / np.log(M3 + 1.0)))) ** np.asarray(
        c_alpha, np.float32
    )[0]

    # ---- shared weight packs ----
    # tw8[p, kc, t] = q8(-64*tw[t, kc*128+p])
    tw8 = np.zeros((128, 2, T), np.float32)
    tw8[0:128, 0, :] = -64.0 * tw[:, 0:128].T
    tw8[0:68, 1, :] = -64.0 * tw[:, 128:196].T
    wntS = np.zeros((128, 3), np.float32)
    wn_t = (tw ** 2).sum(1) + EPS
    wntS[:, :] = 32.0 * wn_t.reshape(3, 128).T
    tb32 = 32.0 * tb.reshape(3, 128).T
    w2s = 2.0 * scale_t * w2  # (P, T)
    w2s8 = np.zeros((128, 4, P), np.float32)
    for kc in range(3):
        w2s8[:, kc, :] = w2s.T[kc * 128 : (kc + 1) * 128, :]
    cw8 = np.zeros((128, 6, M3), np.float32)
    for kc in range(6):
        cw8[:, kc, :] = -64.0 * cw[:, kc * 128 : (kc + 1) * 128].T
    wn_c = (cw ** 2).sum(1) + EPS
    wncS = 32.0 * wn_c.reshape(24, 128).T
    cb32 = 32.0 * cb.reshape(24, 128).T
    w4s = 2.0 * scale_c * w4  # (C, M3)
    w4s8 = np.zeros((128, 24, C), np.float32)
    for mc in range(24):
        w4s8[:, mc, :] = w4s.T[mc * 128 : (mc + 1) * 128, :]
    b4s8 = (64.0 * b4).reshape(1, C)

    shared = {
        "tw8": _q8(tw8),
        "wntS": np.ascontiguousarray(wntS),
        "tb32": np.ascontiguousarray(tb32.astype(np.float32)),
        "w2s8": _q8(w2s8),
        "cw8": _q8(cw8),
        "wncS": np.ascontiguousarray(wncS.astype(np.float32)),
        "cb32": np.ascontiguousarray(cb32.astype(np.float32)),
        "w4s8": _q8(w4s8),
        "b4s8": _q8(b4s8),
    }

    # ---- per-core activations ----
    xr = x.reshape(NCORES, BL, P, C)
    x8 = np.zeros((NCORES, BL, 128, 2, C), np.float32)
    x8[:, :, 0:128, 0, :] = xr[:, :, 0:128, :]
    x8[:, :, 0:68, 1, :] = xr[:, :, 128:196, :]
    x8 = _q8(x8)
    xn1 = (xr.astype(np.float32) ** 2).sum(axis=2)      # (NC, BL, C)
    xn1s = (32.0 * xn1).reshape(NCORES, 1, BL * C).astype(np.float16)
    # xTp[cq, cc, b*196+p] = x[b, p, cc*128+cq] + b2[p]
    xt = xr.transpose(0, 3, 1, 2).reshape(NCORES, C, ROWS) + np.tile(
        b2, BL
    )[None, None, :]
    xTp = xt.reshape(NCORES, 6, 128, ROWS).transpose(0, 2, 1, 3).astype(np.float16)

    in_maps = [
        dict(shared, x8=x8[c], xn1s=xn1s[c], xTp=np.ascontiguousarray(xTp[c]))
        for c in range(NCORES)
    ]

    nc = _get_program()
    kwargs = {}
    if _trace:
        import os
        import shutil

        shutil.rmtree("/tmp/bass_ntff", ignore_errors=True)
        os.makedirs("/tmp/bass_ntff", exist_ok=True)
        kwargs["tmpdir"] = "/tmp/bass_ntff"
    res = bass_utils.run_bass_kernel_spmd(
        nc, in_maps, core_ids=list(range(NCORES)), trace=_trace, **kwargs
    )
    # outT (C, ROWS) fp16 -> (BL, P, C) fp32 per core
    outs = []
    for c in range(NCORES):
        oT = np.asarray(res.results[c]["outT"], np.float32)   # (768, 1568)
        outs.append(oT.reshape(C, BL, P).transpose(1, 2, 0))
    out = np.concatenate(outs, axis=0).reshape(B, P, C)
    if _trace:
        kernel.last_results = res
    return out
